# revision 3
# baseline (speedup 1.0000x reference)
"""MoE (brute-force reference) kernel for 8 TRN2 NeuronCores.

Strategy: expert-parallel. Host routes token-slots by gate_idx to their
expert, pads each expert's slot list to capacity C=256 (overflow slots —
~30 of 3973 for the reference routing — are computed exactly on host),
and transposes so the device sees xt[e] = X_e.T in a partition-major
layout. Each core owns 2 experts and computes
  hT[m] = gelu(sum_k w1T[k,m].T @ xT[k] + b1)   then
  yT[m] = sum_k w2T[k,m].T @ hT[k]
All matmul operands are fp16 (same PE rate as bf16, ~8x the accuracy);
accumulation is fp32 in PSUM. b1 is applied on-device (bias fused into
the gelu activation); b2 and the gate_score combine happen on host in
exact fp32.

Schedule (per core, times relative to the graded window start):
- A short garbage-operand PE warm-up (gpsimd memset, ~8 matmuls) starts
  the HAM clock ramp immediately while the first DMAs stream in, and
  bridges until slab0 + xt k0 land (~+2.4us).
- GEMM1 group 0 (m0..7) runs k-outer so it can start on just slab0 and
  consume slabs just-in-time; its 8 gelus stream on the scalar engine
  while group 1 (m8..15) runs per-m k-inner — each m only needs its own
  PSUM bank back (freed by one gelu), so there is no 8-gelu barrier.
- GEMM2 phase A runs k-outer over k0..7 (banks freed one-by-one by
  group 1's gelus, in the same order); phase B runs per-m k-inner over
  k8..15 so each y[m] completes in turn and its vector-copy eviction +
  scalar-ring DMA stream out during the remaining matmuls.
- The very last output accumulates per column half so half of it evicts
  and DMAs (sync ring) while the other half's matmuls still run; the
  remaining half rides the scalar ring. This cuts the serial
  end-of-kernel chain to ~1.5us.
- All weight DMAs ride the sync HWDGE ring in strict consumption-
  deadline order (slabs-e0, w1b-e0, w2A-e0, w2B-e0, xt-e1, slabs-e1,
  w1b-e1, w2A-e1, w2B-e1); the scalar ring carries only xt-e0 (fine
  k-chunks so GEMM1 can start ASAP) + b1 early, then the y outputs.
  Every transfer is a straight contiguous per-partition copy from a
  host pre-swizzled layout.
"""

import numpy as np

import concourse.bacc as bacc
import concourse.mybir as mybir
from concourse import tile
from concourse.bass_utils import run_bass_kernel_spmd

E, D, H, TOPK, T = 16, 1024, 2048, 2, 2048
NCORES = 8
EPC = E // NCORES  # experts per core
C = 256            # per-expert token capacity after top-k dedup
KD, KH, MD = D // 128, H // 128, D // 128  # 8, 16, 8
KH2 = KH // 2      # 8
HH = H // 2        # GEMM1 column half (m-tiles 0..7 / 8..15)

_F16 = np.float16
_CACHE: dict = {}
_LAST_IN_MAPS = None  # stashed by kernel() for external re-profiling

# xt DMA k-ranges (fine first chunks so GEMM1 m-group0 k0 starts ASAP)
XT_PLAN0 = [(0, 1), (1, 2), (2, 4), (4, 8)]
XT_PLAN1 = [(0, 4), (4, 8)]


def _build(reps: int = 1):
    dt = mybir.dt.float16
    f32 = mybir.dt.float32
    nc = bacc.Bacc("TRN2", target_bir_lowering=False, debug=False,
                   num_devices=NCORES)
    # Host pre-swizzled layouts ([*, 128, free]; every DMA is a straight
    # contiguous per-partition copy):
    # xt: [e][p, k*C + c]            = X_e.T[k*128+p, c]
    # w1: cols 0..KD*HH      slabs   [p, k*HH + c]  = w1T[k*128+p, c]
    #     cols +j*KD*512     Bchunk  [p, k*512+mm*128+c]
    #                                = w1T[k*128+p, HH+j*512+mm*128+c]
    # w2: cols j*4096        Achunk  [p, kk*D + c] = w2T[(4j+kk)*128+p, c]
    #     cols 8192+j*4096   Bchunk  [p, k*512+mm*128+c]
    #                                = w2T[(8+k)*128+p, j*512+mm*128+c]
    xt = nc.dram_tensor("xt", [EPC, 128, KD * C], dt, kind="ExternalInput")
    w1 = nc.dram_tensor("w1", [EPC, 128, D * H // 128], dt,
                        kind="ExternalInput")
    w2 = nc.dram_tensor("w2", [EPC, 128, H * D // 128], dt,
                        kind="ExternalInput")
    b1 = nc.dram_tensor("b1", [EPC, 128, KH], f32, kind="ExternalInput")
    yt = nc.dram_tensor("yt", [EPC, 128, MD * C], dt, kind="ExternalOutput")

    gelu = mybir.ActivationFunctionType.Gelu_apprx_tanh
    WARM = 10  # garbage-operand warm-up matmuls: starts the HAM clock
               # ramp at body start and bridges until slab0+xt0 land

    with tile.TileContext(nc) as tc:
        with (
            tc.tile_pool(name="xtp", bufs=1) as xtp,
            tc.tile_pool(name="wp", bufs=1) as wp,
            tc.tile_pool(name="htp", bufs=1) as htp,
            tc.tile_pool(name="yp", bufs=16) as yp,
            tc.tile_pool(name="bp", bufs=1) as bp,
            tc.tile_pool(name="ps", bufs=1, space="PSUM") as psp,
        ):
            # PE warm-up on a gpsimd-memset tile (gpsimd is idle in the
            # preamble; scalar is blocked by its ACT_TABLE_LOAD).
            wz = bp.tile([128, C], dt, name="warmz", tag="warmz")
            nc.gpsimd.memset(wz[:], 0.0)
            psw = psp.tile([128, C], f32, name="psw", tag="ps0")
            for _ in range(WARM):
                nc.tensor.matmul(psw[:], wz[:, :128], wz[:],
                                 start=True, stop=True)

            for r in range(reps):
                # ---- all input DMAs, per-ring, in consumption-deadline
                # order. Tags are per expert slot (both experts resident;
                # no cross-expert buffer waits anywhere in the stream).
                xts = []   # [e][chunk] -> (tile, ks, ke)
                slabs = []  # [e][i] -> (tile, ks, ke)
                w1b = []   # [e][j] -> tile
                w2a = []   # [e][j] -> tile
                w2b = []   # [e][j] -> tile
                b1s = []
                for e in range(EPC):
                    u = f"{r}_{e}"
                    plan = XT_PLAN0 if e == 0 else XT_PLAN1
                    xts.append([(xtp.tile([128, (ke - ks) * C], dt,
                                          name=f"xt{u}_{i}",
                                          tag=f"xt{e}_{i}"), ks, ke)
                                for i, (ks, ke) in enumerate(plan)])
                    sp = ([(k, k + 1) for k in range(KD)] if e == 0
                          else [(2 * j, 2 * j + 2) for j in range(KD // 2)])
                    slabs.append([(wp.tile([128, (ke - ks) * HH], dt,
                                           name=f"w1a{u}_{i}",
                                           tag=f"w1a{e}_{i}"), ks, ke)
                                  for i, (ks, ke) in enumerate(sp)])
                    w1b.append([wp.tile([128, KD * 512], dt,
                                        name=f"w1b{u}_{j}", tag=f"w1b{e}_{j}")
                                for j in range(2)])
                    w2a.append([wp.tile([128, 4 * D], dt,
                                        name=f"w2a{u}_{j}", tag=f"w2a{e}_{j}")
                                for j in range(2)])
                    w2b.append([wp.tile([128, KH2 * 512], dt,
                                        name=f"w2b{u}_{j}", tag=f"w2b{e}_{j}")
                                for j in range(2)])
                    b1s.append(bp.tile([128, KH], f32, name=f"b1s{u}",
                                       tag=f"b1s{e}"))

                # scalar ring: xt-e0 fine chunks, b1 (tiny), then y later
                for tl, ks, ke in xts[0]:
                    nc.scalar.dma_start(out=tl[:],
                                        in_=xt.ap()[0][:, ks * C:ke * C])
                nc.scalar.dma_start(out=b1s[0][:], in_=b1.ap()[0])
                nc.scalar.dma_start(out=b1s[1][:], in_=b1.ap()[1])

                # sync ring: weights in consumption-deadline order
                def wdma(e_first):
                    order = [0, 1] if e_first == 0 else [1, 0]
                    for idx, e in enumerate(order):
                        for tl, ks, ke in slabs[e]:
                            nc.sync.dma_start(
                                out=tl[:],
                                in_=w1.ap()[e][:, ks * HH:ke * HH])
                        for j in range(2):
                            base = KD * HH + j * KD * 512
                            nc.sync.dma_start(
                                out=w1b[e][j][:],
                                in_=w1.ap()[e][:, base:base + KD * 512])
                        for j in range(2):
                            nc.sync.dma_start(
                                out=w2a[e][j][:],
                                in_=w2.ap()[e][:, j * 4096:(j + 1) * 4096])
                        for j in range(2):
                            base = 8192 + j * 4096
                            nc.sync.dma_start(
                                out=w2b[e][j][:],
                                in_=w2.ap()[e][:, base:base + 4096])
                        if idx == 0 and e_first == 0:
                            # xt-e1 rides the sync ring at its deadline
                            # position (the scalar ring's early window is
                            # reserved for the critical xt-e0 chunks).
                            for tl, ks, ke in xts[1]:
                                nc.sync.dma_start(
                                    out=tl[:],
                                    in_=xt.ap()[1][:, ks * C:ke * C])
                wdma(0)

                def xtv(e, k):
                    for tl, ks, ke in xts[e]:
                        if ks <= k < ke:
                            return tl[:, (k - ks) * C:(k - ks + 1) * C]

                def slabv(e, k, m):
                    for tl, ks, ke in slabs[e]:
                        if ks <= k < ke:
                            off = (k - ks) * HH + m * 128
                            return tl[:, off:off + 128]

                def w1bv(e, k, m):  # m in 8..15
                    j, mm = (m - 8) // 4, (m - 8) % 4
                    off = k * 512 + mm * 128
                    return w1b[e][j][:, off:off + 128]

                def w2av(e, k, m):  # k in 0..7
                    j, kk = k // 4, k % 4
                    off = kk * D + m * 128
                    return w2a[e][j][:, off:off + 128]

                def w2bv(e, k, m):  # k in 8..15
                    j, mm = m // 4, m % 4
                    off = (k - 8) * 512 + mm * 128
                    return w2b[e][j][:, off:off + 128]

                for e in range(EPC):
                    u = f"{r}_{e}"
                    hts = [htp.tile([128, C], dt, name=f"ht{u}_{m}",
                                    tag=f"ht{m}") for m in range(KH)]
                    pss = [psp.tile([128, C], f32, name=f"ps_{u}_{m}",
                                    tag=f"ps{m}") for m in range(MD)]

                    # GEMM1 group 0: k-outer (start on slab0 alone)
                    for k in range(KD):
                        for m in range(MD):
                            nc.tensor.matmul(pss[m][:], slabv(e, k, m),
                                             xtv(e, k), start=(k == 0),
                                             stop=(k == KD - 1))
                    for m in range(MD):
                        nc.scalar.activation(hts[m][:], pss[m][:], gelu,
                                             bias=b1s[e][:, m:m + 1])

                    # GEMM1 group 1: per-m k-inner (each m only waits for
                    # its own bank's gelu, not all eight)
                    for m in range(MD, KH):
                        i = m - MD
                        for k in range(KD):
                            nc.tensor.matmul(pss[i][:], w1bv(e, k, m),
                                             xtv(e, k), start=(k == 0),
                                             stop=(k == KD - 1))
                        nc.scalar.activation(hts[m][:], pss[i][:], gelu,
                                             bias=b1s[e][:, m:m + 1])

                    # GEMM2 phase A: k-outer over k0..7 (banks freed
                    # one-by-one by group 1's gelus, in the same order)
                    for k in range(KH2):
                        for m in range(MD):
                            nc.tensor.matmul(pss[m][:], w2av(e, k, m),
                                             hts[k][:], start=(k == 0),
                                             stop=False)

                    # GEMM2 phase B: per-m k-inner; evict + DMA per m
                    last = (r == reps - 1 and e == EPC - 1)
                    CH = C // 2
                    for m in range(MD):
                        yo = yp.tile([128, C], dt, name=f"y{u}_{m}", tag="y")
                        if last and m == MD - 1:
                            for k in range(KH2, KH):
                                nc.tensor.matmul(
                                    pss[m][:, :CH], w2bv(e, k, m),
                                    hts[k][:, :CH],
                                    start=False, stop=(k == KH - 1))
                            nc.vector.tensor_copy(out=yo[:, :CH],
                                                  in_=pss[m][:, :CH])
                            nc.sync.dma_start(
                                out=yt.ap()[e][:, m * C:m * C + CH],
                                in_=yo[:, :CH])
                            for k in range(KH2, KH):
                                nc.tensor.matmul(
                                    pss[m][:, CH:], w2bv(e, k, m),
                                    hts[k][:, CH:],
                                    start=False, stop=(k == KH - 1))
                            nc.scalar.activation(
                                yo[:, CH:], pss[m][:, CH:],
                                mybir.ActivationFunctionType.Copy)
                            nc.scalar.dma_start(
                                out=yt.ap()[e][:, m * C + CH:(m + 1) * C],
                                in_=yo[:, CH:])
                        else:
                            for k in range(KH2, KH):
                                nc.tensor.matmul(pss[m][:], w2bv(e, k, m),
                                                 hts[k][:], start=False,
                                                 stop=(k == KH - 1))
                            nc.vector.tensor_copy(out=yo[:], in_=pss[m][:])
                            y_eng = nc.sync if (last and m % 2 == 1) \
                                else nc.scalar
                            y_eng.dma_start(
                                out=yt.ap()[e][:, m * C:(m + 1) * C],
                                in_=yo[:])
    nc.compile()
    return nc


def _get_nc(reps: int = 1):
    if reps not in _CACHE:
        _CACHE[reps] = _build(reps)
    return _CACHE[reps]


def _route(gate_idx, gate_score):
    """Dedup routing: tokens whose two top-k picks are the same expert are
    sent once with summed score. Returns per-expert (tokens, weights,
    overflow_tokens, overflow_weights)."""
    g = np.asarray(gate_idx).astype(np.int64)
    sc = np.asarray(gate_score, dtype=np.float32)
    out = []
    for e in range(E):
        m0, m1 = g[:, 0] == e, g[:, 1] == e
        toks = np.flatnonzero(m0 | m1)
        wts = (sc[:, 0] * m0 + sc[:, 1] * m1)[toks]
        out.append((toks[:C], wts[:C], toks[C:], wts[C:]))
    return out


def kernel(inp, gate_idx, gate_score, w1, b1, w2, b2):
    inp = np.asarray(inp, dtype=np.float32)
    gate_idx = np.asarray(gate_idx)
    gate_score = np.asarray(gate_score, dtype=np.float32)
    w1 = np.asarray(w1, dtype=np.float32)
    b1 = np.asarray(b1, dtype=np.float32)
    w2 = np.asarray(w2, dtype=np.float32)
    b2 = np.asarray(b2, dtype=np.float32)

    routes = _route(gate_idx, gate_score)

    # Host-side gather + swizzle into the device layouts, fp16.
    xt_all = np.zeros((E, 128, KD, C), dtype=_F16)
    for e in range(E):
        toks = routes[e][0]
        n = len(toks)
        if n:
            xt_all[e, :, :, :n] = (
                inp[toks].T.reshape(KD, 128, n).transpose(1, 0, 2)
                .astype(_F16))
    xt_all = xt_all.reshape(E, 128, KD * C)

    # w1: slabs (cols 0..HH) then 2 per-m B-chunks (cols HH..2HH).
    w1t = np.ascontiguousarray(w1.transpose(0, 2, 1)).astype(_F16)  # [E,D,H]
    a = (w1t[:, :, :HH].reshape(E, KD, 128, HH).transpose(0, 2, 1, 3)
         .reshape(E, 128, KD * HH))
    bs = [w1t[:, :, HH + j * 512:HH + (j + 1) * 512]
          .reshape(E, KD, 128, 512).transpose(0, 2, 1, 3)
          .reshape(E, 128, KD * 512) for j in range(2)]
    w1d = np.ascontiguousarray(np.concatenate([a] + bs, axis=2))

    # w2: 2 A-chunks (k0..7, k-outer) then 2 per-m B-chunks (k8..15).
    w2t = np.ascontiguousarray(w2.transpose(0, 2, 1)).astype(_F16)  # [E,H,D]
    a2 = [w2t[:, j * 512:(j + 1) * 512, :].reshape(E, 4, 128, D)
          .transpose(0, 2, 1, 3).reshape(E, 128, 4 * D) for j in range(2)]
    b2c = [w2t[:, HH:, j * 512:(j + 1) * 512]
           .reshape(E, KH2, 128, 512).transpose(0, 2, 1, 3)
           .reshape(E, 128, KH2 * 512) for j in range(2)]
    w2d = np.ascontiguousarray(np.concatenate(a2 + b2c, axis=2))

    in_maps = []
    for c in range(NCORES):
        sl = slice(EPC * c, EPC * (c + 1))
        in_maps.append({
            "xt": xt_all[sl],
            "w1": w1d[sl],
            "w2": w2d[sl],
            "b1": np.ascontiguousarray(
                b1[sl].reshape(EPC, KH, 128).transpose(0, 2, 1)),
        })

    global _LAST_IN_MAPS
    _LAST_IN_MAPS = in_maps

    nc = _get_nc()
    res = run_bass_kernel_spmd(nc, in_maps, list(range(NCORES)))

    # Host combine: weight each expert's output columns by the (summed)
    # gate score and accumulate per token; add the b2 term (folded out of
    # the device kernel). Tokens are unique within an expert, so the
    # fancy-indexed += is safe.
    out = np.einsum("tk,tkd->td", np.asarray(gate_score, dtype=np.float32),
                    b2[np.asarray(gate_idx).astype(np.int64)])
    out = np.ascontiguousarray(out, dtype=np.float32)
    for e in range(E):
        core, le = divmod(e, EPC)
        toks, wts, otoks, owts = routes[e]
        if len(toks):
            ytr = res.results[core]["yt"][le].reshape(128, MD, C)
            y = (ytr.transpose(1, 0, 2).reshape(D, C)[:, :len(toks)]
                 .T.astype(np.float32))
            out[toks] += wts[:, None] * y
        if len(otoks):  # exact host fallback for capacity overflow
            hh = inp[otoks] @ w1[e].T + b1[e]
            hh = 0.5 * hh * (1.0 + np.tanh(
                np.sqrt(2.0 / np.pi) * (hh + 0.044715 * hh ** 3)))
            out[otoks] += owts[:, None] * (hh @ w2[e].T)
    return out


# revision 9
# speedup vs baseline: 1.0473x; 1.0473x over previous
"""MoE (brute-force reference) kernel for 8 TRN2 NeuronCores.

Strategy: expert-parallel. Host routes token-slots by gate_idx to their
expert, pads each expert's slot list to capacity C=256 (overflow slots —
~30 of 3973 for the reference routing — are computed exactly on host),
and transposes so the device sees xt[e] = X_e.T in a partition-major
layout. Each core owns 2 experts and computes
  hT[m] = gelu(sum_k w1T[k,m].T @ xT[k] + b1)   then
  yT[m] = sum_k w2T[k,m].T @ hT[k]
All matmul operands are fp16 (same PE rate as bf16, ~8x the accuracy);
accumulation is fp32 in PSUM. b1 is applied on-device (bias fused into
the gelu activation); b2 and the gate_score combine happen on host in
exact fp32.

Schedule (per core, times relative to the graded window start):
- A short garbage-operand PE warm-up (gpsimd memset, ~8 matmuls) starts
  the HAM clock ramp immediately while the first DMAs stream in, and
  bridges until slab0 + xt k0 land (~+2.4us).
- GEMM1 group 0 (m0..7) runs k-outer so it can start on just slab0 and
  consume slabs just-in-time; its 8 gelus stream on the scalar engine
  while group 1 (m8..15) runs per-m k-inner — each m only needs its own
  PSUM bank back (freed by one gelu), so there is no 8-gelu barrier.
- GEMM2 phase A runs k-outer over k0..7 (banks freed one-by-one by
  group 1's gelus, in the same order); phase B runs per-m k-inner over
  k8..15 so each y[m] completes in turn and its vector-copy eviction +
  scalar-ring DMA stream out during the remaining matmuls.
- The very last output accumulates per column half so half of it evicts
  and DMAs (sync ring) while the other half's matmuls still run; the
  remaining half rides the scalar ring. This cuts the serial
  end-of-kernel chain to ~1.5us.
- All weight DMAs ride the sync HWDGE ring in strict consumption-
  deadline order (slabs-e0, w1b-e0, w2A-e0, w2B-e0, xt-e1, slabs-e1,
  w1b-e1, w2A-e1, w2B-e1); the scalar ring carries only xt-e0 (fine
  k-chunks so GEMM1 can start ASAP) + b1 early, then the y outputs.
  Every transfer is a straight contiguous per-partition copy from a
  host pre-swizzled layout.
"""

import numpy as np

import concourse.bacc as bacc
import concourse.mybir as mybir
from concourse import tile
from concourse.bass_utils import run_bass_kernel_spmd

E, D, H, TOPK, T = 16, 1024, 2048, 2, 2048
NCORES = 8
EPC = E // NCORES  # experts per core
C = 256            # per-expert token capacity after top-k dedup
KD, KH, MD = D // 128, H // 128, D // 128  # 8, 16, 8
KH2 = KH // 2      # 8
HH = H // 2        # GEMM1 column half (m-tiles 0..7 / 8..15)

_F16 = np.float16
_CACHE: dict = {}
_LAST_IN_MAPS = None  # stashed by kernel() for external re-profiling

# xt DMA k-ranges. e0 stays at TWO chunks: the scalar ring must carry as
# few early DMAs as possible — every early DMA takes one of the ~10
# shared DMA semaphore lanes, and a reused lane makes the ISSUING engine
# wait for the prior DMA on that lane to complete, which tangles the
# sync-ring weight chain behind scalar transfers.
XT_PLAN0 = [(0, 2), (2, 8)]
XT_PLAN1 = [(0, 8)]


def _build(reps: int = 1):
    dt = mybir.dt.float16
    f32 = mybir.dt.float32
    nc = bacc.Bacc("TRN2", target_bir_lowering=False, debug=False,
                   num_devices=NCORES)
    # Host pre-swizzled layouts ([*, 128, free]; every DMA is a straight
    # contiguous per-partition copy):
    # xt: [e][p, k*C + c]            = X_e.T[k*128+p, c]
    # w1: cols 0..KD*HH      slabs   [p, k*HH + c]  = w1T[k*128+p, c]
    #     cols +j*KD*512     Bchunk  [p, k*512+mm*128+c]
    #                                = w1T[k*128+p, HH+j*512+mm*128+c]
    # w2: cols j*4096        Achunk  [p, kk*D + c] = w2T[(4j+kk)*128+p, c]
    #     cols 8192+j*4096   Bchunk  [p, k*512+mm*128+c]
    #                                = w2T[(8+k)*128+p, j*512+mm*128+c]
    xt = nc.dram_tensor("xt", [EPC, 128, KD * C], dt, kind="ExternalInput")
    w1 = nc.dram_tensor("w1", [EPC, 128, D * H // 128], dt,
                        kind="ExternalInput")
    w2 = nc.dram_tensor("w2", [EPC, 128, H * D // 128], dt,
                        kind="ExternalInput")
    b1 = nc.dram_tensor("b1", [EPC, 128, KH], f32, kind="ExternalInput")
    yt = nc.dram_tensor("yt", [EPC, 128, MD * C], dt, kind="ExternalOutput")

    gelu = mybir.ActivationFunctionType.Gelu_apprx_tanh
    WARM = 10  # garbage-operand warm-up matmuls: starts the HAM clock
               # ramp at body start and bridges until slab0+xt0 land

    with tile.TileContext(nc) as tc:
        with (
            tc.tile_pool(name="xtp", bufs=1) as xtp,
            tc.tile_pool(name="wp", bufs=1) as wp,
            tc.tile_pool(name="htp", bufs=1) as htp,
            tc.tile_pool(name="yp", bufs=16) as yp,
            tc.tile_pool(name="bp", bufs=1) as bp,
            tc.tile_pool(name="ps", bufs=1, space="PSUM") as psp,
        ):
            # PE warm-up on a gpsimd-memset tile (gpsimd is idle in the
            # preamble; scalar is blocked by its ACT_TABLE_LOAD).
            wz = bp.tile([128, C], dt, name="warmz", tag="warmz")
            nc.gpsimd.memset(wz[:], 0.0)
            psw = psp.tile([128, C], f32, name="psw", tag="ps0")
            for _ in range(WARM):
                nc.tensor.matmul(psw[:], wz[:, :128], wz[:],
                                 start=True, stop=True)

            for r in range(reps):
                # ---- all input DMAs, per-ring, in consumption-deadline
                # order. Tags are per expert slot (both experts resident;
                # no cross-expert buffer waits anywhere in the stream).
                xts = []   # [e][chunk] -> (tile, ks, ke)
                slabs = []  # [e][i] -> (tile, ks, ke)
                w1b = []   # [e][j] -> tile
                w2a = []   # [e][j] -> tile
                w2b = []   # [e][j] -> tile
                b1s = []
                for e in range(EPC):
                    u = f"{r}_{e}"
                    plan = XT_PLAN0 if e == 0 else XT_PLAN1
                    xts.append([(xtp.tile([128, (ke - ks) * C], dt,
                                          name=f"xt{u}_{i}",
                                          tag=f"xt{e}_{i}"), ks, ke)
                                for i, (ks, ke) in enumerate(plan)])
                    sp = ([(k, k + 1) for k in range(KD)] if e == 0
                          else [(2 * j, 2 * j + 2) for j in range(KD // 2)])
                    slabs.append([(wp.tile([128, (ke - ks) * HH], dt,
                                           name=f"w1a{u}_{i}",
                                           tag=f"w1a{e}_{i}"), ks, ke)
                                  for i, (ks, ke) in enumerate(sp)])
                    # w1b: 4 logical chunks of 2 m-tiles ([128, KD*256],
                    # [p, k*256 + mm*128 + c]); e0 DMAs them singly for
                    # just-in-time per-m arrival, e1 pairs them up.
                    nb = 4 if e == 0 else 2
                    w1b.append([wp.tile([128, KD * 1024 // nb], dt,
                                        name=f"w1b{u}_{j}", tag=f"w1b{e}_{j}")
                                for j in range(nb)])
                    w2a.append([wp.tile([128, 4 * D], dt,
                                        name=f"w2a{u}_{j}", tag=f"w2a{e}_{j}")
                                for j in range(2)])
                    w2b.append([wp.tile([128, KH2 * 512], dt,
                                        name=f"w2b{u}_{j}", tag=f"w2b{e}_{j}")
                                for j in range(2)])
                    b1s.append(bp.tile([128, KH], f32, name=f"b1s{u}",
                                       tag=f"b1s{e}"))

                # scalar ring early: ONLY the two xt-e0 chunks (keeps the
                # shared DMA sem lanes clear of cross-ring dependencies)
                for tl, ks, ke in xts[0]:
                    nc.scalar.dma_start(out=tl[:],
                                        in_=xt.ap()[0][:, ks * C:ke * C])

                # sync ring: everything else, strict consumption-deadline
                # order: slabs-e0, b1-e0, w1b-e0, w2a-e0, w2b-e0, xt-e1,
                # slabs-e1, b1-e1, w1b-e1, w2a-e1, w2b-e1.
                def wsync(e):
                    for tl, ks, ke in slabs[e]:
                        nc.sync.dma_start(
                            out=tl[:], in_=w1.ap()[e][:, ks * HH:ke * HH])
                    nc.sync.dma_start(out=b1s[e][:], in_=b1.ap()[e])
                    nb = len(w1b[e])
                    w = KD * 1024 // nb
                    for j in range(nb):
                        base = KD * HH + j * w
                        nc.sync.dma_start(out=w1b[e][j][:],
                                          in_=w1.ap()[e][:, base:base + w])
                    for j in range(2):
                        nc.sync.dma_start(
                            out=w2a[e][j][:],
                            in_=w2.ap()[e][:, j * 4096:(j + 1) * 4096])
                    for j in range(2):
                        base = 8192 + j * 4096
                        nc.sync.dma_start(out=w2b[e][j][:],
                                          in_=w2.ap()[e][:, base:base + 4096])

                wsync(0)
                for tl, ks, ke in xts[1]:
                    nc.sync.dma_start(out=tl[:],
                                      in_=xt.ap()[1][:, ks * C:ke * C])
                wsync(1)

                def xtv(e, k):
                    for tl, ks, ke in xts[e]:
                        if ks <= k < ke:
                            return tl[:, (k - ks) * C:(k - ks + 1) * C]

                def slabv(e, k, m):
                    for tl, ks, ke in slabs[e]:
                        if ks <= k < ke:
                            off = (k - ks) * HH + m * 128
                            return tl[:, off:off + 128]

                def w1bv(e, k, m):  # m in 8..15
                    j, mm = (m - 8) // 2, (m - 8) % 2
                    cic = k * 256 + mm * 128
                    if len(w1b[e]) == 4:
                        return w1b[e][j][:, cic:cic + 128]
                    off = (j % 2) * KD * 256 + cic
                    return w1b[e][j // 2][:, off:off + 128]

                def w2av(e, k, m):  # k in 0..7
                    j, kk = k // 4, k % 4
                    off = kk * D + m * 128
                    return w2a[e][j][:, off:off + 128]

                def w2bv(e, k, m):  # k in 8..15
                    j, mm = m // 4, m % 4
                    off = (k - 8) * 512 + mm * 128
                    return w2b[e][j][:, off:off + 128]

                for e in range(EPC):
                    u = f"{r}_{e}"
                    hts = [htp.tile([128, C], dt, name=f"ht{u}_{m}",
                                    tag=f"ht{m}") for m in range(KH)]
                    pss = [psp.tile([128, C], f32, name=f"ps_{u}_{m}",
                                    tag=f"ps{m}") for m in range(MD)]

                    # GEMM1 group 0: k-outer (start on slab0 alone)
                    for k in range(KD):
                        for m in range(MD):
                            nc.tensor.matmul(pss[m][:], slabv(e, k, m),
                                             xtv(e, k), start=(k == 0),
                                             stop=(k == KD - 1))
                    for m in range(MD):
                        nc.scalar.activation(hts[m][:], pss[m][:], gelu,
                                             bias=b1s[e][:, m:m + 1])

                    # GEMM1 group 1: per-m k-inner (each m only waits for
                    # its own bank's gelu, not all eight)
                    for m in range(MD, KH):
                        i = m - MD
                        for k in range(KD):
                            nc.tensor.matmul(pss[i][:], w1bv(e, k, m),
                                             xtv(e, k), start=(k == 0),
                                             stop=(k == KD - 1))
                        nc.scalar.activation(hts[m][:], pss[i][:], gelu,
                                             bias=b1s[e][:, m:m + 1])

                    # GEMM2 phase A: k-outer over k0..7 (banks freed
                    # one-by-one by group 1's gelus, in the same order)
                    for k in range(KH2):
                        for m in range(MD):
                            nc.tensor.matmul(pss[m][:], w2av(e, k, m),
                                             hts[k][:], start=(k == 0),
                                             stop=False)

                    # GEMM2 phase B: per-m k-inner; evict + DMA per m
                    last = (r == reps - 1 and e == EPC - 1)
                    CH = C // 2
                    for m in range(MD):
                        yo = yp.tile([128, C], dt, name=f"y{u}_{m}", tag="y")
                        if last and m == MD - 1:
                            for k in range(KH2, KH):
                                nc.tensor.matmul(
                                    pss[m][:, :CH], w2bv(e, k, m),
                                    hts[k][:, :CH],
                                    start=False, stop=(k == KH - 1))
                            nc.vector.tensor_copy(out=yo[:, :CH],
                                                  in_=pss[m][:, :CH])
                            nc.sync.dma_start(
                                out=yt.ap()[e][:, m * C:m * C + CH],
                                in_=yo[:, :CH])
                            for k in range(KH2, KH):
                                nc.tensor.matmul(
                                    pss[m][:, CH:], w2bv(e, k, m),
                                    hts[k][:, CH:],
                                    start=False, stop=(k == KH - 1))
                            nc.scalar.activation(
                                yo[:, CH:], pss[m][:, CH:],
                                mybir.ActivationFunctionType.Copy)
                            nc.scalar.dma_start(
                                out=yt.ap()[e][:, m * C + CH:(m + 1) * C],
                                in_=yo[:, CH:])
                        else:
                            for k in range(KH2, KH):
                                nc.tensor.matmul(pss[m][:], w2bv(e, k, m),
                                                 hts[k][:], start=False,
                                                 stop=(k == KH - 1))
                            nc.vector.tensor_copy(out=yo[:], in_=pss[m][:])
                            y_eng = nc.sync if (last and m % 2 == 1) \
                                else nc.scalar
                            y_eng.dma_start(
                                out=yt.ap()[e][:, m * C:(m + 1) * C],
                                in_=yo[:])
    nc.compile()
    return nc


def _get_nc(reps: int = 1):
    if reps not in _CACHE:
        _CACHE[reps] = _build(reps)
    return _CACHE[reps]


def _route(gate_idx, gate_score):
    """Dedup routing: tokens whose two top-k picks are the same expert are
    sent once with summed score. Returns per-expert (tokens, weights,
    overflow_tokens, overflow_weights)."""
    g = np.asarray(gate_idx).astype(np.int64)
    sc = np.asarray(gate_score, dtype=np.float32)
    out = []
    for e in range(E):
        m0, m1 = g[:, 0] == e, g[:, 1] == e
        toks = np.flatnonzero(m0 | m1)
        wts = (sc[:, 0] * m0 + sc[:, 1] * m1)[toks]
        out.append((toks[:C], wts[:C], toks[C:], wts[C:]))
    return out


def kernel(inp, gate_idx, gate_score, w1, b1, w2, b2):
    inp = np.asarray(inp, dtype=np.float32)
    gate_idx = np.asarray(gate_idx)
    gate_score = np.asarray(gate_score, dtype=np.float32)
    w1 = np.asarray(w1, dtype=np.float32)
    b1 = np.asarray(b1, dtype=np.float32)
    w2 = np.asarray(w2, dtype=np.float32)
    b2 = np.asarray(b2, dtype=np.float32)

    routes = _route(gate_idx, gate_score)

    # Host-side gather + swizzle into the device layouts, fp16.
    xt_all = np.zeros((E, 128, KD, C), dtype=_F16)
    for e in range(E):
        toks = routes[e][0]
        n = len(toks)
        if n:
            xt_all[e, :, :, :n] = (
                inp[toks].T.reshape(KD, 128, n).transpose(1, 0, 2)
                .astype(_F16))
    xt_all = xt_all.reshape(E, 128, KD * C)

    # w1: slabs (cols 0..HH) then 2 per-m B-chunks (cols HH..2HH).
    w1t = np.ascontiguousarray(w1.transpose(0, 2, 1)).astype(_F16)  # [E,D,H]
    a = (w1t[:, :, :HH].reshape(E, KD, 128, HH).transpose(0, 2, 1, 3)
         .reshape(E, 128, KD * HH))
    bs = [w1t[:, :, HH + j * 256:HH + (j + 1) * 256]
          .reshape(E, KD, 128, 256).transpose(0, 2, 1, 3)
          .reshape(E, 128, KD * 256) for j in range(4)]
    w1d = np.ascontiguousarray(np.concatenate([a] + bs, axis=2))

    # w2: 2 A-chunks (k0..7, k-outer) then 2 per-m B-chunks (k8..15).
    w2t = np.ascontiguousarray(w2.transpose(0, 2, 1)).astype(_F16)  # [E,H,D]
    a2 = [w2t[:, j * 512:(j + 1) * 512, :].reshape(E, 4, 128, D)
          .transpose(0, 2, 1, 3).reshape(E, 128, 4 * D) for j in range(2)]
    b2c = [w2t[:, HH:, j * 512:(j + 1) * 512]
           .reshape(E, KH2, 128, 512).transpose(0, 2, 1, 3)
           .reshape(E, 128, KH2 * 512) for j in range(2)]
    w2d = np.ascontiguousarray(np.concatenate(a2 + b2c, axis=2))

    in_maps = []
    for c in range(NCORES):
        sl = slice(EPC * c, EPC * (c + 1))
        in_maps.append({
            "xt": xt_all[sl],
            "w1": w1d[sl],
            "w2": w2d[sl],
            "b1": np.ascontiguousarray(
                b1[sl].reshape(EPC, KH, 128).transpose(0, 2, 1)),
        })

    global _LAST_IN_MAPS
    _LAST_IN_MAPS = in_maps

    nc = _get_nc()
    res = run_bass_kernel_spmd(nc, in_maps, list(range(NCORES)))

    # Host combine: weight each expert's output columns by the (summed)
    # gate score and accumulate per token; add the b2 term (folded out of
    # the device kernel). Tokens are unique within an expert, so the
    # fancy-indexed += is safe.
    out = np.einsum("tk,tkd->td", np.asarray(gate_score, dtype=np.float32),
                    b2[np.asarray(gate_idx).astype(np.int64)])
    out = np.ascontiguousarray(out, dtype=np.float32)
    for e in range(E):
        core, le = divmod(e, EPC)
        toks, wts, otoks, owts = routes[e]
        if len(toks):
            ytr = res.results[core]["yt"][le].reshape(128, MD, C)
            y = (ytr.transpose(1, 0, 2).reshape(D, C)[:, :len(toks)]
                 .T.astype(np.float32))
            out[toks] += wts[:, None] * y
        if len(otoks):  # exact host fallback for capacity overflow
            hh = inp[otoks] @ w1[e].T + b1[e]
            hh = 0.5 * hh * (1.0 + np.tanh(
                np.sqrt(2.0 / np.pi) * (hh + 0.044715 * hh ** 3)))
            out[otoks] += owts[:, None] * (hh @ w2[e].T)
    return out
